# revision 1
# baseline (speedup 1.0000x reference)
"""Trainium2 Bass kernel for nn_BagKQMClassModel.

Computation (per batch item b):
    K[b,n,m]   = exp(-d2/(2 s^2)),  d2 = |A[b,n] - C[m]|^2
    out_w[b,m] = (1/N) sum_n comp_w[m] * K^2
    y_w        = out_w / sum_m out_w
    probs      = y_w @ (y_v^2),  y_v = c_y rows normalized

Key transformations used here:
  * K^2 = exp(-d2/s^2), so only one exp per (b,n,m) element is needed.
  * d2 = a2[bn] + b2[m] - 2 g[m,bn] with g = C @ A^T.  Both -a2/2 and -b2/2
    are folded into the matmul by augmenting the contraction dim (K=34):
        row 32: C^T row = 1,       A^T row = -a2/2
        row 33: C^T row = -b2/2,   A^T row = 1
    so one matmul emits g_full with exp argument = (2/s^2) * g_full and the
    activation needs no bias at all.
  * probs = T[:, :10] / T[:, 10] where T[b,:] = sum_n sum_m K2[m,bn]*W[m,:],
    W[m, :10] = comp_w[m] * c_y[m]^2 / |c_y[m]|^2,  W[m, 10] = comp_w[m].
    The 1/N bag weight and normalization cancel.
  * Layout (m on partitions, b*n on free dim): matmul2 contracts m on the PE
    with W as the stationary operand; the final n-reduction happens on the
    tiny (11, bn) result via a segmented DVE reduce.

Sharding: batch 256 -> 32 items per core across 8 cores; c_x/c_y/comp_w
replicated. No collectives (forward only).
"""

import numpy as np

import concourse.bacc as bacc
import concourse.mybir as mybir
import concourse.tile as tile
from concourse.bass import ts
from concourse.bass_utils import run_bass_kernel_spmd
from concourse.masks import make_identity

NCORES = 8
BS, N, DX, DY, M = 256, 128, 32, 10, 2048
BPC = BS // NCORES      # 32 batch items per core
MB = M // 128           # 16 chunks of the component axis
KAUG = DX + 2           # 34: contraction dim with the two folded rows
NBLK = 4                # bn blocks per core
BLKI = BPC // NBLK      # 8 items per block
F_BLK = BLKI * N        # 1024 free elements per (block, m-chunk) tile
MIN_SIGMA = 1e-3
FP32 = mybir.dt.float32
BF16 = mybir.dt.bfloat16
FP16 = mybir.dt.float16
AX = mybir.AxisListType
ALU = mybir.AluOpType
ACTF = mybir.ActivationFunctionType


def _body(tc, inp, cx, cy, cw_d, out_d, scale):
    nc = tc.nc
    from contextlib import ExitStack

    with ExitStack() as ctx:
        const = ctx.enter_context(tc.tile_pool(name="const", bufs=1))
        work = ctx.enter_context(tc.tile_pool(name="work", bufs=2))
        k2p = ctx.enter_context(tc.tile_pool(name="k2p", bufs=4))
        psum = ctx.enter_context(tc.tile_pool(name="psum", bufs=2, space="PSUM"))

        identity = const.tile([128, 128], FP32)
        make_identity(nc, identity)
        identity_bf = const.tile([128, 128], FP16)
        make_identity(nc, identity_bf)

        # ---- input loads ---------------------------------------------------
        # c_x / inputs land in augmented (128, 34) layouts: cols 0:32 = data,
        # plus the 1.0 and -b2/2 (resp. -a2/2) columns, so transposing whole
        # tiles yields the augmented C^T/A^T rows without partition-offset
        # writes.  The gather DMAs are 128B-granule (slow), so they are
        # chunked to overlap with the per-chunk prep chains.
        cx_aug = const.tile([128, MB, KAUG], FP32)
        cx_r = cx.rearrange("(t p) d -> p t d", p=128)
        for c in range(2):
            nc.scalar.dma_start(
                out=cx_aug[:, ts(c, MB // 2), 0:DX], in_=cx_r[:, ts(c, MB // 2), :]
            )
        A_aug = const.tile([128, BPC, KAUG], FP32)
        inp_r = inp.rearrange("t p d -> p t d")
        for c in range(4):
            eng = nc.sync if c < 2 else nc.scalar
            eng.dma_start(
                out=A_aug[:, ts(c, BPC // 4), 0:DX], in_=inp_r[:, ts(c, BPC // 4), :]
            )
        nc.vector.memset(A_aug[:, :, DX + 1 : DX + 2], 1.0)
        nc.vector.memset(cx_aug[:, :, DX : DX + 1], 1.0)
        cy_all = const.tile([128, MB, DY], FP32)
        nc.gpsimd.dma_start(out=cy_all, in_=cy.rearrange("(t p) d -> p t d", p=128))
        cw_sb = const.tile([128, MB], FP32)
        nc.gpsimd.dma_start(out=cw_sb, in_=cw_d.rearrange("(t p) -> p t", p=128))

        # bf16 for both matmul operands: fp32 matmuls stream at 1/4 PE rate.
        CT = const.tile([KAUG, M], FP16)      # augmented C^T (stationary mm1)
        AT = const.tile([KAUG, BPC, N], FP16)  # augmented A^T (moving mm1)
        W_all = const.tile([128, MB, DY + 1], FP32)
        W_bf = const.tile([128, MB, DY + 1], BF16)
        T_sb = const.tile([DY + 1, BPC], FP32)


        # ---- chunked prep: square/reduce -> bf16 cast -> transposes -------
        # (fp16 transposes stream 4x faster through the PE than fp32 ones;
        # all psum->sbuf copies go to the DVE: the ACT engine is the kernel's
        # floor, so any copy there delays the exp stream 1:1)
        cx_bf = const.tile([128, MB, KAUG], FP16)
        A_bf = const.tile([128, BPC, KAUG], FP16)

        def prep_chunk(aug, bft, sq_tag, lo, hi, col):
            sqc = work.tile([128, hi - lo, DX], FP32, tag=sq_tag)
            nc.vector.tensor_mul(sqc, aug[:, lo:hi, 0:DX], aug[:, lo:hi, 0:DX])
            nc.vector.tensor_reduce(
                out=aug[:, lo:hi, col : col + 1], in_=sqc, axis=AX.X, op=ALU.add
            )
            nc.vector.tensor_scalar_mul(
                aug[:, lo:hi, col : col + 1], aug[:, lo:hi, col : col + 1], -0.5
            )
            nc.vector.tensor_copy(bft[:, lo:hi, :], aug[:, lo:hi, :])

        def transpose_to(bft, dst_is_ct, lo, hi):
            for k in range(lo, hi):
                trk = psum.tile([KAUG, 128], FP16, tag="jit", bufs=4)
                nc.tensor.transpose(trk, bft[:, k, :], identity_bf)
                dst = CT[:, ts(k, 128)] if dst_is_ct else AT[:, k, :]
                if k % 2 == 0:
                    nc.scalar.copy(dst, trk)
                else:
                    nc.vector.tensor_copy(dst, trk)

        for c in range(2):
            lo, hi = c * (MB // 2), (c + 1) * (MB // 2)
            prep_chunk(cx_aug, cx_bf, "sqx", lo, hi, DX + 1)
            transpose_to(cx_bf, True, lo, hi)
        # ---- W build (gates mm2 of the first loop step; emitted before the
        # A transposes so the DVE finishes it well ahead of the main loop) --
        sqy = work.tile([128, MB, DY], FP32, tag="sqy")
        nc.vector.tensor_mul(sqy, cy_all, cy_all)
        ssum = work.tile([128, MB], FP32, tag="ssum")
        nc.vector.tensor_reduce(out=ssum, in_=sqy, axis=AX.X, op=ALU.add)
        rec = work.tile([128, MB], FP32, tag="rec")
        nc.vector.reciprocal(rec, ssum)
        facr = work.tile([128, MB], FP32, tag="facr")
        nc.vector.tensor_mul(facr, rec, cw_sb)
        facr_b = facr.rearrange("p (t one) -> p t one", one=1).broadcast_to(
            [128, MB, DY]
        )
        nc.vector.tensor_mul(W_all[:, :, 0:DY], sqy, facr_b)
        nc.vector.tensor_copy(
            W_all[:, :, DY : DY + 1], cw_sb.rearrange("p (t one) -> p t one", one=1)
        )
        nc.vector.tensor_copy(W_bf, W_all)

        for c in range(4):
            lo, hi = c * (BPC // 4), (c + 1) * (BPC // 4)
            prep_chunk(A_aug, A_bf, "sqa", lo, hi, DX)
            transpose_to(A_bf, False, lo, hi)


        # ---- main pipeline ------------------------------------------------
        # The PE clock is power-limited to 1.2 GHz when all 8 cores run, so
        # the loop is balanced for that operating point: the DVE pre-reduces
        # K2 4:1 over n-pairs (valid because mm2 is linear in its free dim),
        # shrinking mm2's PE stream from 1024 to 256 columns per step.  Cold
        # PE (~1.2us) then matches the ACT exp (~1.1us) instead of pacing the
        # loop at ~1.8us.  mm2/reduce for step mb-1 are issued after mm1 for
        # step mb so the PE FIFO never blocks on the current step's ACT.
        F_R2 = F_BLK // 8

        def emit_reduce(k2):
            r1 = work.tile([128, F_BLK // 2], BF16, tag="r1")
            k2v = k2.rearrange("p (t two n) -> p t two n", two=2, n=N // 2)
            nc.vector.tensor_add(
                r1.rearrange("p (t n) -> p t n", n=N // 2),
                k2v[:, :, 0, :],
                k2v[:, :, 1, :],
            )
            r2 = work.tile([128, F_BLK // 4], BF16, tag="r2")
            r1v = r1.rearrange("p (t two n) -> p t two n", two=2, n=N // 4)
            nc.vector.tensor_add(
                r2.rearrange("p (t n) -> p t n", n=N // 4),
                r1v[:, :, 0, :],
                r1v[:, :, 1, :],
            )
            r3 = work.tile([128, F_R2], BF16, tag="r3")
            r2v = r2.rearrange("p (t two n) -> p t two n", two=2, n=N // 8)
            nc.vector.tensor_add(
                r3.rearrange("p (t n) -> p t n", n=N // 8),
                r2v[:, :, 0, :],
                r2v[:, :, 1, :],
            )
            return r3

        def emit_blk_reduce(S, blk):
            nc.vector.tensor_reduce(
                out=T_sb[:, blk * BLKI : (blk + 1) * BLKI],
                in_=S.rearrange("p (t n) -> p t n", n=N // 8),
                axis=AX.X,
                op=ALU.add,
            )

        # Flat loop: the mm2 software pipeline (one step behind mm1/exp)
        # carries across block boundaries so the engines never drain.
        r2_prev = None
        S_tiles = [None] * NBLK
        for j in range(NBLK * MB):
            blk, mb = divmod(j, MB)
            if mb == 0:
                S_tiles[blk] = psum.tile(
                    [DY + 1, F_R2], FP32, tag="jit", bufs=4, name=f"S{blk}"
                )
            g = psum.tile([128, F_BLK], FP32, tag="g", bufs=2)
            for q in range(F_BLK // 512):
                nc.tensor.matmul(
                    g[:, ts(q, 512)],
                    CT[:, ts(mb, 128)],
                    AT[:, blk * BLKI + q * 4 : blk * BLKI + (q + 1) * 4, :],
                    start=True,
                    stop=True,
                )
            if r2_prev is not None:
                pblk, pmb = divmod(j - 1, MB)
                nc.tensor.matmul(
                    S_tiles[pblk],
                    W_bf[:, pmb, :],
                    r2_prev,
                    start=(pmb == 0),
                    stop=(pmb == MB - 1),
                )
                if pmb == MB - 1:
                    emit_blk_reduce(S_tiles[pblk], pblk)
            K2 = k2p.tile([128, F_BLK], BF16, tag="k2")
            nc.scalar.activation(K2, g, ACTF.Exp, bias=0.0, scale=scale)
            r2_prev = emit_reduce(K2)
        last_blk, last_mb = NBLK - 1, MB - 1
        nc.tensor.matmul(
            S_tiles[last_blk], W_bf[:, last_mb, :], r2_prev, start=False, stop=True
        )
        emit_blk_reduce(S_tiles[last_blk], last_blk)

        # ---- epilogue: probs = T[:, :10] / T[:, 10] -----------------------
        trT = psum.tile([BPC, DY + 1], FP32, tag="jit", bufs=4)
        nc.tensor.transpose(trT, T_sb, identity[0 : DY + 1, 0 : DY + 1])
        Tt = const.tile([BPC, DY + 1], FP32)
        nc.vector.tensor_copy(Tt, trT)
        recd = const.tile([BPC, 1], FP32)
        nc.vector.reciprocal(recd, Tt[:, DY : DY + 1])
        outsb = const.tile([BPC, DY], FP32)
        nc.vector.tensor_scalar(
            out=outsb, in0=Tt[:, 0:DY], scalar1=recd, scalar2=None, op0=ALU.mult
        )
        nc.sync.dma_start(out=out_d, in_=outsb)


def build_program(scale):
    nc = bacc.Bacc(
        "TRN2",
        target_bir_lowering=False,
        debug=False,
        enable_asserts=False,
        num_devices=NCORES,
    )
    inp = nc.dram_tensor("inputs", [BPC, N, DX], FP32, kind="ExternalInput").ap()
    cx = nc.dram_tensor("c_x", [M, DX], FP32, kind="ExternalInput").ap()
    cy = nc.dram_tensor("c_y", [M, DY], FP32, kind="ExternalInput").ap()
    cw = nc.dram_tensor("comp_w", [M], FP32, kind="ExternalInput").ap()
    out = nc.dram_tensor("out", [BPC, DY], FP32, kind="ExternalOutput").ap()
    with tile.TileContext(nc) as tc:
        _body(tc, inp, cx, cy, cw, out, scale)
    nc.compile()
    return nc


_PROGRAM_CACHE: dict = {}


def _get_program(scale):
    nc = _PROGRAM_CACHE.get(scale)
    if nc is None:
        nc = build_program(scale)
        _PROGRAM_CACHE[scale] = nc
    return nc


def make_in_maps(inputs, c_x, c_y, comp_w):
    shards = np.ascontiguousarray(inputs.reshape(NCORES, BPC, N, DX))
    return [
        {
            "inputs": shards[i],
            "c_x": np.ascontiguousarray(c_x),
            "c_y": np.ascontiguousarray(c_y),
            "comp_w": np.ascontiguousarray(comp_w),
        }
        for i in range(NCORES)
    ]


def scale_from_sigma(sigma) -> float:
    s = max(float(np.asarray(sigma, dtype=np.float64)), MIN_SIGMA)
    return float(2.0 / (s * s))


def kernel(inputs, sigma, c_x, c_y, comp_w, _run_kwargs=None):
    nc = _get_program(scale_from_sigma(sigma))
    in_maps = make_in_maps(inputs, c_x, c_y, comp_w)
    res = run_bass_kernel_spmd(
        nc, in_maps, core_ids=list(range(NCORES)), **(_run_kwargs or {})
    )
    out = np.concatenate([res.results[i]["out"] for i in range(NCORES)], axis=0)
    return out.astype(np.float32)

